# revision 14
# baseline (speedup 1.0000x reference)
"""TRN2 Bass kernel for nn_Mix2Layer (dense MLP mixture).

Reference computation (all fp32):
    g   = relu(einsum('bi,iok->bok', x, w1) + b1)        # [B, DOUT, K]
    out = einsum('bi,iok,bok->bo', x, w2, g) + b2        # [B, DOUT]

Strategy: 2x4 grid over the 8 NeuronCores — batch B split in 2 groups of
1024 rows, DOUT split in 4 shards of 512 (the bok intermediate never
leaves its core). On each core both einsums are plain matmuls of the
core's x rows [1024, DIN] against the shard's weights flattened to
[DIN, DS*K], run on the PE array in float32r — the PE fast path for
4-byte floats (1 cycle/row when the moving dim is >=256, i.e.
bf16-speed). float32r keeps 11 explicit mantissa bits (measured on
hardware: round-to-nearest-even at 11 bits on both operands reproduces
the PE result to 1e-7), giving ~2e-4 relative error overall.

All operands are pre-rounded to the fp32r grid and PACKED on the HOST
into per-tile contiguous blocks (xT: one 1 MB block per b-tile, w1/w2:
one 4 MB block per ok-chunk), so every DMA reads 8-32 KB contiguous per
partition at HBM line rate. All input DMAs go through the sync-engine
HWDGE ring in consumption order — a second concurrent DMA stream
(scalar ring) measurably starves (~70 MB/s) against the main stream and
stalled the PE for tens of us waiting on xT tiles.

Inner loop (ch=512 columns of the flattened DS*K=8192, h1 staged through
SBUF to halve the matmul instruction count — N=512 moving dim amortizes
the ~11 ns per-matmul issue overhead measured at N=256):
  for ok_chunk (16 x 512 cols):
    phase A: for b_tile (8): psum_h1 = sum_i xT_i.T @ w1_chunk_i
             h1s[b_tile] = relu(psum_h1)          (ScalarE -> SBUF)
    phase B: for b_tile (8): psum_h2 = sum_i xT_i.T @ w2_chunk_i
             p = h1s[b_tile] * psum_h2            (VectorE, one fused op)
             acc[b_tile][:, chunk] = reduce_k(p)  (VectorE, 3D-AP reduce)
  DMA acc tiles -> out rows
"""
import numpy as np

import concourse.bass as bass
import concourse.tile as tile
import concourse.mybir as mybir
from concourse import bacc
from concourse.bass_interp import get_hw_module
from concourse.bass_utils import run_bass_kernel_spmd

P = 128
f32 = mybir.dt.float32
f32r = mybir.dt.float32r

N_CORES = 8
B_GROUPS = 2   # batch split across cores
D_GROUPS = 4   # dout split across cores
F32R_MANT_BITS = 11


def round_f32r(a):
    """Round fp32 array to the fp32r grid (11 explicit mantissa bits, RNE)."""
    a = np.ascontiguousarray(a, dtype=np.float32)
    bits = a.view(np.uint32)
    shift = np.uint32(23 - F32R_MANT_BITS)
    lsb = np.uint32(1) << shift
    half = lsb >> np.uint32(1)
    rounded = (bits + half + ((bits >> shift) & np.uint32(1)) - np.uint32(1)) \
        & ~np.uint32(lsb - np.uint32(1))
    return rounded.view(np.float32)


def build_program(din, b, dout_s, k, with_b1, with_b2, ch=512,
                  num_devices=N_CORES):
    """Build + schedule + compile the per-core Bass program.

    din: contraction dim; b: per-core batch rows; dout_s: per-core dout
    shard; k: mixture. ch: ok-chunk width (matmul free dim).
    """
    okw = dout_s * k
    assert din % P == 0 and b % P == 0 and okw % ch == 0
    assert ch % k == 0
    it_n = din // P
    nbt = b // P
    nch = okw // ch
    o_ch = ch // k

    nc = bacc.Bacc("TRN2", target_bir_lowering=False, debug=False,
                   enable_asserts=True, num_devices=num_devices)
    # Host-packed layouts: one contiguous [P, cols] block per tile.
    xt_d = nc.dram_tensor("xtp", [nbt, P, it_n * P], f32r,
                          kind="ExternalInput").ap()
    w1_d = nc.dram_tensor("w1p", [nch, P, it_n * ch], f32r,
                          kind="ExternalInput").ap()
    w2_d = nc.dram_tensor("w2p", [nch, P, it_n * ch], f32r,
                          kind="ExternalInput").ap()
    b1_d = nc.dram_tensor("b1s", [okw], f32r, kind="ExternalInput").ap()
    b2_d = nc.dram_tensor("b2s", [dout_s], f32, kind="ExternalInput").ap()
    out_d = nc.dram_tensor("out", [b, dout_s], f32, kind="ExternalOutput").ap()

    from contextlib import ExitStack
    with tile.TileContext(nc) as tc, ExitStack() as ctx:
        # PE warm-up: the HAM clock gate holds the PE at 1.2 GHz until it
        # sees ~3.4 us of sustained activity, and the first real matmul
        # can't start until ~30 us in (xt0 + w1c0 DMA delivery). Without
        # this, the first ~18 matmuls run at half clock (~3.6 us lost).
        # Dependency-free scratch matmuls during the DMA preamble keep
        # the PE busy so the real stream starts at full 2.4 GHz. The
        # DRAM-scratch DMA at the end anchors the chain against DCE.
        warm_sb = ctx.enter_context(tc.tile_pool(name="warm", bufs=1))
        warm_ps = ctx.enter_context(
            tc.tile_pool(name="wps", bufs=1, space="PSUM"))
        warm_dr = ctx.enter_context(
            tc.tile_pool(name="wdr", bufs=1, space="DRAM"))
        wsrc = warm_sb.tile([P, P], f32, tag="wsrc")
        nc.any.memset(wsrc[:], 0.0)
        wdst = warm_ps.tile([P, 64], f32, tag="wps")
        for _ in range(240):
            nc.tensor.matmul(wdst[:], lhsT=wsrc[:, :P], rhs=wsrc[:, :64],
                             start=True, stop=True)
        wsink = warm_sb.tile([P, 64], f32, tag="wsink")
        nc.scalar.copy(wsink[:], wdst[:])
        # gpsimd ring: otherwise idle here — a sync/scalar-ring DMA would
        # sit at that ring's FIFO head waiting on the warm matmuls and
        # block the real loads queued behind it.
        wscratch = warm_dr.tile([P, 64], f32, tag="wscr")
        nc.gpsimd.dma_start(wscratch[:], wsink[:])

        xt_pool = ctx.enter_context(tc.tile_pool(name="xt", bufs=nbt))
        # Single shared-tag ring: at most one weight DMA in flight at a
        # time behind the current pair — two concurrent DMA write streams
        # into SBUF degrade the PE issue rate (120 -> 144 ns measured).
        w_pool = ctx.enter_context(tc.tile_pool(name="w", bufs=3))
        h1_pool = ctx.enter_context(tc.tile_pool(name="h1", bufs=nbt))
        acc_pool = ctx.enter_context(tc.tile_pool(name="acc", bufs=nbt))
        ep_pool = ctx.enter_context(tc.tile_pool(name="ep", bufs=3))
        const_pool = ctx.enter_context(tc.tile_pool(name="const", bufs=1))
        ps1_pool = ctx.enter_context(
            tc.tile_pool(name="ps1", bufs=2, space="PSUM"))
        ps2_pool = ctx.enter_context(
            tc.tile_pool(name="ps2", bufs=2, space="PSUM"))

        if with_b1:
            ones_t = const_pool.tile([1, P], f32r, tag="ones")
            nc.any.memset(ones_t[:], 1.0)
        if with_b2:
            b2bc = const_pool.tile([P, dout_s], f32, tag="b2bc")
            nc.gpsimd.dma_start(b2bc[:],
                                b2_d[None, :].broadcast_to([P, dout_s]))

        # Issue order on the single sync ring follows first-use order:
        # xt0, w1c0, xt1..7, w2c0, then the steady chunk stream. Delivery
        # (~2.8 us per xt tile at line rate) stays ahead of phase-A
        # consumption (~3.6 us per b-tile group).
        xts = [xt_pool.tile([P, it_n * P], f32r, tag="xtb", name=f"xt_{i}")
               for i in range(nbt)]
        nc.sync.dma_start(xts[0][:], xt_d[0])

        w1_t0 = w_pool.tile([P, it_n * ch], f32r, tag="w")
        nc.sync.dma_start(w1_t0[:], w1_d[0])
        for bt in range(1, nbt):
            nc.sync.dma_start(xts[bt][:], xt_d[bt])
        w2_t0 = w_pool.tile([P, it_n * ch], f32r, tag="w")
        nc.sync.dma_start(w2_t0[:], w2_d[0])

        h1s = [h1_pool.tile([P, ch], f32, tag="h1s", name=f"h1_{i}")
               for i in range(nbt)]
        accs = [acc_pool.tile([P, dout_s], f32, tag="acc",
                              name=f"acc_{i}")
                for i in range(nbt)]

        for c in range(nch):
            if c == 0:
                w1_t, w2_t = w1_t0, w2_t0
            else:
                w1_t = w_pool.tile([P, it_n * ch], f32r, tag="w")
                nc.sync.dma_start(w1_t[:], w1_d[c])
                w2_t = w_pool.tile([P, it_n * ch], f32r, tag="w")
                nc.sync.dma_start(w2_t[:], w2_d[c])
            if with_b1:
                b1r = ep_pool.tile([1, ch], f32r, tag="b1r")
                nc.sync.dma_start(
                    b1r[:], b1_d[None, c * ch:(c + 1) * ch])

            # phase A: h1 = relu(x @ w1chunk) for all b-tiles
            for bt in range(nbt):
                ph1 = ps1_pool.tile([P, ch], f32, tag="ph1")
                for it in range(it_n):
                    nc.tensor.matmul(
                        ph1[:],
                        lhsT=xts[bt][:, it * P:(it + 1) * P],
                        rhs=w1_t[:, it * ch:(it + 1) * ch],
                        start=(it == 0),
                        stop=(it == it_n - 1 and not with_b1),
                    )
                if with_b1:
                    nc.tensor.matmul(ph1[:], lhsT=ones_t[:], rhs=b1r[:],
                                     start=False, stop=True)
                nc.scalar.activation(
                    h1s[bt][:], ph1[:], mybir.ActivationFunctionType.Relu)

            # phase B: h2 = x @ w2chunk; acc[:, chunk] = reduce_k(h1 * h2)
            for bt in range(nbt):
                ph2 = ps2_pool.tile([P, ch], f32, tag="ph2")
                for it in range(it_n):
                    nc.tensor.matmul(
                        ph2[:],
                        lhsT=xts[bt][:, it * P:(it + 1) * P],
                        rhs=w2_t[:, it * ch:(it + 1) * ch],
                        start=(it == 0),
                        stop=(it == it_n - 1),
                    )
                p_t = ep_pool.tile([P, ch], f32, tag="pt")
                nc.vector.scalar_tensor_tensor(
                    out=p_t[:], in0=ph2[:], scalar=0.0, in1=h1s[bt][:],
                    op0=mybir.AluOpType.add, op1=mybir.AluOpType.mult)
                nc.vector.tensor_reduce(
                    out=accs[bt][:, c * o_ch:(c + 1) * o_ch],
                    in_=p_t[:].rearrange("p (o k) -> p o k", k=k),
                    axis=mybir.AxisListType.X,
                    op=mybir.AluOpType.add)

        for bt in range(nbt):
            if with_b2:
                nc.vector.tensor_add(accs[bt][:], accs[bt][:], b2bc[:])
            nc.scalar.dma_start(
                out_d[bt * P:(bt + 1) * P, :],
                accs[bt][:])

    nc.compile()
    nc.m = get_hw_module(nc.m)
    return nc


def _pack_xt(x_rows, it_n):
    """[b, din] fp32r rows -> [nbt, P, it_n*P]: per-b-tile contiguous,
    partition-major, so each tile is one line-rate DMA."""
    b, din = x_rows.shape
    nbt = b // P
    a = x_rows.reshape(nbt, P, it_n, P)          # [bt, bl, it, pi]
    a = a.transpose(0, 3, 2, 1)                  # [bt, pi, it, bl]
    return np.ascontiguousarray(a.reshape(nbt, P, it_n * P))


def _pack_w(w_flat, it_n, ch):
    """[din, okw] fp32r -> [nch, P, it_n*ch]: per-chunk contiguous."""
    din, okw = w_flat.shape
    nch = okw // ch
    a = w_flat.reshape(it_n, P, nch, ch)         # [it, p, c, j]
    a = a.transpose(2, 1, 0, 3)                  # [c, p, it, j]
    return np.ascontiguousarray(a.reshape(nch, P, it_n * ch))


CH = 512


def shard_inputs(x, w1, b1, w2, b2, n_cores=N_CORES):
    b_dim, din = x.shape
    _, dout, k = w1.shape
    bs = b_dim // B_GROUPS
    ds = dout // D_GROUPS
    it_n = din // P
    xr = round_f32r(np.asarray(x, np.float32))
    xts = [_pack_xt(xr[r * bs:(r + 1) * bs], it_n) for r in range(B_GROUPS)]
    w1s = [_pack_w(round_f32r(w1[:, c * ds:(c + 1) * ds, :])
                   .reshape(din, ds * k), it_n, CH)
           for c in range(D_GROUPS)]
    w2s = [_pack_w(round_f32r(w2[:, c * ds:(c + 1) * ds, :])
                   .reshape(din, ds * k), it_n, CH)
           for c in range(D_GROUPS)]
    b1s = [round_f32r(b1[c * ds:(c + 1) * ds, :]).reshape(ds * k)
           for c in range(D_GROUPS)]
    b2s = [np.ascontiguousarray(b2[c * ds:(c + 1) * ds], dtype=np.float32)
           for c in range(D_GROUPS)]
    in_maps = []
    for cid in range(n_cores):
        r, c = divmod(cid, D_GROUPS)
        in_maps.append({
            "xtp": xts[r],
            "w1p": w1s[c],
            "w2p": w2s[c],
            "b1s": b1s[c],
            "b2s": b2s[c],
        })
    return in_maps


def unshard_output(results, b_dim, dout):
    bs = b_dim // B_GROUPS
    ds = dout // D_GROUPS
    out = np.empty((b_dim, dout), dtype=np.float32)
    for cid in range(N_CORES):
        r, c = divmod(cid, D_GROUPS)
        out[r * bs:(r + 1) * bs, c * ds:(c + 1) * ds] = results[cid]["out"]
    return out


_PROGRAM_CACHE = {}


def _get_program(din, b, dout_s, k, with_b1, with_b2):
    key = (din, b, dout_s, k, with_b1, with_b2, CH)
    if key not in _PROGRAM_CACHE:
        _PROGRAM_CACHE[key] = build_program(
            din, b, dout_s, k, with_b1, with_b2, ch=CH)
    return _PROGRAM_CACHE[key]


class ParallelRunner:
    """Dispatch the per-core NEFF to each NeuronCore via its own jit so the
    8 executions overlap. (run_bass_kernel_spmd's shard_map path serializes
    the per-device executes through the axon proxy — measured 8x slower
    wall-clock for identical device work.)"""

    def __init__(self, nc, n_cores=N_CORES):
        import jax
        from concourse import bass2jax
        bass2jax.install_neuronx_cc_hook()
        self.jax = jax
        self.n_cores = n_cores
        part = nc.partition_id_tensor.name if nc.partition_id_tensor else None

        in_names, out_names, out_avals, zero_outs = [], [], [], []
        for alloc in nc.m.functions[0].allocations:
            if not isinstance(alloc, mybir.MemoryLocationSet):
                continue
            name = alloc.memorylocations[0].name
            if alloc.kind == "ExternalInput":
                if name != part:
                    in_names.append(name)
            elif alloc.kind == "ExternalOutput":
                out_names.append(name)
                shape = tuple(alloc.tensor_shape)
                dtype = mybir.dt.np(alloc.dtype)
                out_avals.append(jax.core.ShapedArray(shape, dtype))
                zero_outs.append(np.zeros(shape, dtype))
        self.in_names, self.out_names = in_names, out_names
        all_names = in_names + out_names + ([part] if part else [])

        def _body(*args):
            operands = list(args)
            if part is not None:
                operands.append(bass2jax.partition_id_tensor())
            return tuple(bass2jax._bass_exec_p.bind(
                *operands,
                out_avals=tuple(out_avals),
                in_names=tuple(all_names),
                out_names=tuple(out_names),
                lowering_input_output_aliases=(),
                sim_require_finite=True,
                sim_require_nnan=True,
                nc=nc,
            ))

        self.devices = jax.devices()[:n_cores]
        self.fns = [jax.jit(_body, device=d, keep_unused=True)
                    for d in self.devices]
        self.zero_dev = [
            [jax.device_put(z, d) for z in zero_outs] for d in self.devices]

    def __call__(self, in_maps):
        outs = []
        for c in range(self.n_cores):
            args = [self.jax.device_put(np.asarray(in_maps[c][n]),
                                        self.devices[c])
                    for n in self.in_names]
            outs.append(self.fns[c](*args, *self.zero_dev[c]))
        self.jax.block_until_ready(outs)
        return [{n: np.asarray(outs[c][i])
                 for i, n in enumerate(self.out_names)}
                for c in range(self.n_cores)]


_RUNNER_CACHE = {}


def _run(nc, in_maps):
    key = id(nc)
    try:
        if key not in _RUNNER_CACHE:
            _RUNNER_CACHE[key] = ParallelRunner(nc)
        return _RUNNER_CACHE[key](in_maps)
    except Exception:
        res = run_bass_kernel_spmd(nc, in_maps,
                                   core_ids=list(range(N_CORES)))
        return res.results


def kernel(x, w1, b1, w2, b2):
    x = np.asarray(x, dtype=np.float32)
    w1 = np.asarray(w1, dtype=np.float32)
    b1 = np.asarray(b1, dtype=np.float32)
    w2 = np.asarray(w2, dtype=np.float32)
    b2 = np.asarray(b2, dtype=np.float32)

    b_dim, din = x.shape
    _, dout, k = w1.shape
    bs = b_dim // B_GROUPS
    ds = dout // D_GROUPS

    nc = _get_program(din, bs, ds, k,
                      bool(np.any(b1)), bool(np.any(b2)))
    in_maps = shard_inputs(x, w1, b1, w2, b2)
    results = _run(nc, in_maps)
    return np.ascontiguousarray(unshard_output(results, b_dim, dout))


# revision 15
# speedup vs baseline: 1.0841x; 1.0841x over previous
"""TRN2 Bass kernel for nn_Mix2Layer (dense MLP mixture).

Reference computation (all fp32):
    g   = relu(einsum('bi,iok->bok', x, w1) + b1)        # [B, DOUT, K]
    out = einsum('bi,iok,bok->bo', x, w2, g) + b2        # [B, DOUT]

Strategy: 2x4 grid over the 8 NeuronCores — batch B split in 2 groups of
1024 rows, DOUT split in 4 shards of 512 (the bok intermediate never
leaves its core). On each core both einsums are plain matmuls of the
core's x rows [1024, DIN] against the shard's weights flattened to
[DIN, DS*K], run on the PE array in float32r — the PE fast path for
4-byte floats (1 cycle/row when the moving dim is >=256, i.e.
bf16-speed). float32r keeps 11 explicit mantissa bits (measured on
hardware: round-to-nearest-even at 11 bits on both operands reproduces
the PE result to 1e-7), giving ~2e-4 relative error overall.

All operands are pre-rounded to the fp32r grid and PACKED on the HOST
into per-tile contiguous blocks (xT: one 1 MB block per b-tile, w1/w2:
one 4 MB block per ok-chunk), so every DMA reads 8-32 KB contiguous per
partition at HBM line rate. All input DMAs go through the sync-engine
HWDGE ring in consumption order — a second concurrent DMA stream
(scalar ring) measurably starves (~70 MB/s) against the main stream and
stalled the PE for tens of us waiting on xT tiles.

Inner loop (ch=512 columns of the flattened DS*K=8192, h1 staged through
SBUF to halve the matmul instruction count — N=512 moving dim amortizes
the ~11 ns per-matmul issue overhead measured at N=256):
  for ok_chunk (16 x 512 cols):
    phase A: for b_tile (8): psum_h1 = sum_i xT_i.T @ w1_chunk_i
             h1s[b_tile] = relu(psum_h1)          (ScalarE -> SBUF)
    phase B: for b_tile (8): psum_h2 = sum_i xT_i.T @ w2_chunk_i
             p = h1s[b_tile] * psum_h2            (VectorE, one fused op)
             acc[b_tile][:, chunk] = reduce_k(p)  (VectorE, 3D-AP reduce)
  DMA acc tiles -> out rows
"""
import numpy as np

import concourse.bass as bass
import concourse.tile as tile
import concourse.mybir as mybir
from concourse import bacc
from concourse.bass_interp import get_hw_module
from concourse.bass_utils import run_bass_kernel_spmd

P = 128
f32 = mybir.dt.float32
f32r = mybir.dt.float32r

N_CORES = 8
B_GROUPS = 2   # batch split across cores
D_GROUPS = 4   # dout split across cores
F32R_MANT_BITS = 11


def round_f32r(a):
    """Round fp32 array to the fp32r grid (11 explicit mantissa bits, RNE)."""
    a = np.ascontiguousarray(a, dtype=np.float32)
    bits = a.view(np.uint32)
    shift = np.uint32(23 - F32R_MANT_BITS)
    lsb = np.uint32(1) << shift
    half = lsb >> np.uint32(1)
    rounded = (bits + half + ((bits >> shift) & np.uint32(1)) - np.uint32(1)) \
        & ~np.uint32(lsb - np.uint32(1))
    return rounded.view(np.float32)


def build_program(din, b, dout_s, k, with_b1, with_b2, ch=512,
                  num_devices=N_CORES):
    """Build + schedule + compile the per-core Bass program.

    din: contraction dim; b: per-core batch rows; dout_s: per-core dout
    shard; k: mixture. ch: ok-chunk width (matmul free dim).
    """
    okw = dout_s * k
    assert din % P == 0 and b % P == 0 and okw % ch == 0
    assert ch % k == 0
    it_n = din // P
    nbt = b // P
    nch = okw // ch
    o_ch = ch // k

    nc = bacc.Bacc("TRN2", target_bir_lowering=False, debug=False,
                   enable_asserts=True, num_devices=num_devices)
    # Host-packed layouts: one contiguous [P, cols] block per tile.
    xt_d = nc.dram_tensor("xtp", [nbt, P, it_n * P], f32r,
                          kind="ExternalInput").ap()
    w1_d = nc.dram_tensor("w1p", [nch, P, it_n * ch], f32r,
                          kind="ExternalInput").ap()
    w2_d = nc.dram_tensor("w2p", [nch, P, it_n * ch], f32r,
                          kind="ExternalInput").ap()
    b1_d = nc.dram_tensor("b1s", [okw], f32r, kind="ExternalInput").ap()
    b2_d = nc.dram_tensor("b2s", [dout_s], f32, kind="ExternalInput").ap()
    out_d = nc.dram_tensor("out", [b, dout_s], f32, kind="ExternalOutput").ap()

    from contextlib import ExitStack
    with tile.TileContext(nc) as tc, ExitStack() as ctx:
        # PE warm-up: the HAM clock gate holds the PE at 1.2 GHz until it
        # sees ~3.4 us of sustained activity, and the first real matmul
        # can't start until ~30 us in (xt0 + w1c0 DMA delivery). Without
        # this, the first ~18 matmuls run at half clock (~3.6 us lost).
        # Dependency-free scratch matmuls during the DMA preamble keep
        # the PE busy so the real stream starts at full 2.4 GHz. The
        # DRAM-scratch DMA at the end anchors the chain against DCE.
        warm_sb = ctx.enter_context(tc.tile_pool(name="warm", bufs=1))
        warm_ps = ctx.enter_context(
            tc.tile_pool(name="wps", bufs=1, space="PSUM"))
        warm_dr = ctx.enter_context(
            tc.tile_pool(name="wdr", bufs=1, space="DRAM"))
        wsrc = warm_sb.tile([P, P], f32, tag="wsrc")
        nc.any.memset(wsrc[:], 0.0)
        wdst = warm_ps.tile([P, 64], f32, tag="wps")
        # 48 pairs x ~360 ns (plain-fp32 pair rate) ~= 17 us: bridges the
        # PE from the ~11 us engine barrier to the ~29 us first real
        # matmul. More would push past the preamble and delay the stream.
        for _ in range(48):
            nc.tensor.matmul(wdst[:], lhsT=wsrc[:, :P], rhs=wsrc[:, :64],
                             start=True, stop=True)
        wsink = warm_sb.tile([P, 64], f32, tag="wsink")
        nc.scalar.copy(wsink[:], wdst[:])
        # gpsimd ring: otherwise idle here — a sync/scalar-ring DMA would
        # sit at that ring's FIFO head waiting on the warm matmuls and
        # block the real loads queued behind it.
        wscratch = warm_dr.tile([P, 64], f32, tag="wscr")
        nc.gpsimd.dma_start(wscratch[:], wsink[:])

        xt_pool = ctx.enter_context(tc.tile_pool(name="xt", bufs=nbt))
        # Single shared-tag ring: at most one weight DMA in flight at a
        # time behind the current pair — two concurrent DMA write streams
        # into SBUF degrade the PE issue rate (120 -> 144 ns measured).
        w_pool = ctx.enter_context(tc.tile_pool(name="w", bufs=3))
        h1_pool = ctx.enter_context(tc.tile_pool(name="h1", bufs=nbt))
        acc_pool = ctx.enter_context(tc.tile_pool(name="acc", bufs=nbt))
        ep_pool = ctx.enter_context(tc.tile_pool(name="ep", bufs=3))
        const_pool = ctx.enter_context(tc.tile_pool(name="const", bufs=1))
        ps1_pool = ctx.enter_context(
            tc.tile_pool(name="ps1", bufs=2, space="PSUM"))
        ps2_pool = ctx.enter_context(
            tc.tile_pool(name="ps2", bufs=2, space="PSUM"))

        if with_b1:
            ones_t = const_pool.tile([1, P], f32r, tag="ones")
            nc.any.memset(ones_t[:], 1.0)
        if with_b2:
            b2bc = const_pool.tile([P, dout_s], f32, tag="b2bc")
            nc.gpsimd.dma_start(b2bc[:],
                                b2_d[None, :].broadcast_to([P, dout_s]))

        # Issue order on the single sync ring follows first-use order:
        # xt0, w1c0, xt1..7, w2c0, then the steady chunk stream. Delivery
        # (~2.8 us per xt tile at line rate) stays ahead of phase-A
        # consumption (~3.6 us per b-tile group).
        xts = [xt_pool.tile([P, it_n * P], f32r, tag="xtb", name=f"xt_{i}")
               for i in range(nbt)]
        nc.sync.dma_start(xts[0][:], xt_d[0])

        w1_t0 = w_pool.tile([P, it_n * ch], f32r, tag="w")
        nc.sync.dma_start(w1_t0[:], w1_d[0])
        for bt in range(1, nbt):
            nc.sync.dma_start(xts[bt][:], xt_d[bt])
        w2_t0 = w_pool.tile([P, it_n * ch], f32r, tag="w")
        nc.sync.dma_start(w2_t0[:], w2_d[0])

        h1s = [h1_pool.tile([P, ch], f32, tag="h1s", name=f"h1_{i}")
               for i in range(nbt)]
        accs = [acc_pool.tile([P, dout_s], f32, tag="acc",
                              name=f"acc_{i}")
                for i in range(nbt)]

        for c in range(nch):
            if c == 0:
                w1_t, w2_t = w1_t0, w2_t0
            else:
                w1_t = w_pool.tile([P, it_n * ch], f32r, tag="w")
                nc.sync.dma_start(w1_t[:], w1_d[c])
                w2_t = w_pool.tile([P, it_n * ch], f32r, tag="w")
                nc.sync.dma_start(w2_t[:], w2_d[c])
            if with_b1:
                b1r = ep_pool.tile([1, ch], f32r, tag="b1r")
                nc.sync.dma_start(
                    b1r[:], b1_d[None, c * ch:(c + 1) * ch])

            # phase A: h1 = relu(x @ w1chunk) for all b-tiles
            for bt in range(nbt):
                ph1 = ps1_pool.tile([P, ch], f32, tag="ph1")
                for it in range(it_n):
                    nc.tensor.matmul(
                        ph1[:],
                        lhsT=xts[bt][:, it * P:(it + 1) * P],
                        rhs=w1_t[:, it * ch:(it + 1) * ch],
                        start=(it == 0),
                        stop=(it == it_n - 1 and not with_b1),
                    )
                if with_b1:
                    nc.tensor.matmul(ph1[:], lhsT=ones_t[:], rhs=b1r[:],
                                     start=False, stop=True)
                nc.scalar.activation(
                    h1s[bt][:], ph1[:], mybir.ActivationFunctionType.Relu)

            # phase B: h2 = x @ w2chunk; acc[:, chunk] = reduce_k(h1 * h2)
            for bt in range(nbt):
                ph2 = ps2_pool.tile([P, ch], f32, tag="ph2")
                for it in range(it_n):
                    nc.tensor.matmul(
                        ph2[:],
                        lhsT=xts[bt][:, it * P:(it + 1) * P],
                        rhs=w2_t[:, it * ch:(it + 1) * ch],
                        start=(it == 0),
                        stop=(it == it_n - 1),
                    )
                p_t = ep_pool.tile([P, ch], f32, tag="pt")
                nc.vector.scalar_tensor_tensor(
                    out=p_t[:], in0=ph2[:], scalar=0.0, in1=h1s[bt][:],
                    op0=mybir.AluOpType.add, op1=mybir.AluOpType.mult)
                nc.vector.tensor_reduce(
                    out=accs[bt][:, c * o_ch:(c + 1) * o_ch],
                    in_=p_t[:].rearrange("p (o k) -> p o k", k=k),
                    axis=mybir.AxisListType.X,
                    op=mybir.AluOpType.add)

        for bt in range(nbt):
            if with_b2:
                nc.vector.tensor_add(accs[bt][:], accs[bt][:], b2bc[:])
            nc.scalar.dma_start(
                out_d[bt * P:(bt + 1) * P, :],
                accs[bt][:])

    nc.compile()
    nc.m = get_hw_module(nc.m)
    return nc


def _pack_xt(x_rows, it_n):
    """[b, din] fp32r rows -> [nbt, P, it_n*P]: per-b-tile contiguous,
    partition-major, so each tile is one line-rate DMA."""
    b, din = x_rows.shape
    nbt = b // P
    a = x_rows.reshape(nbt, P, it_n, P)          # [bt, bl, it, pi]
    a = a.transpose(0, 3, 2, 1)                  # [bt, pi, it, bl]
    return np.ascontiguousarray(a.reshape(nbt, P, it_n * P))


def _pack_w(w_flat, it_n, ch):
    """[din, okw] fp32r -> [nch, P, it_n*ch]: per-chunk contiguous."""
    din, okw = w_flat.shape
    nch = okw // ch
    a = w_flat.reshape(it_n, P, nch, ch)         # [it, p, c, j]
    a = a.transpose(2, 1, 0, 3)                  # [c, p, it, j]
    return np.ascontiguousarray(a.reshape(nch, P, it_n * ch))


CH = 512


def shard_inputs(x, w1, b1, w2, b2, n_cores=N_CORES):
    b_dim, din = x.shape
    _, dout, k = w1.shape
    bs = b_dim // B_GROUPS
    ds = dout // D_GROUPS
    it_n = din // P
    xr = round_f32r(np.asarray(x, np.float32))
    xts = [_pack_xt(xr[r * bs:(r + 1) * bs], it_n) for r in range(B_GROUPS)]
    w1s = [_pack_w(round_f32r(w1[:, c * ds:(c + 1) * ds, :])
                   .reshape(din, ds * k), it_n, CH)
           for c in range(D_GROUPS)]
    w2s = [_pack_w(round_f32r(w2[:, c * ds:(c + 1) * ds, :])
                   .reshape(din, ds * k), it_n, CH)
           for c in range(D_GROUPS)]
    b1s = [round_f32r(b1[c * ds:(c + 1) * ds, :]).reshape(ds * k)
           for c in range(D_GROUPS)]
    b2s = [np.ascontiguousarray(b2[c * ds:(c + 1) * ds], dtype=np.float32)
           for c in range(D_GROUPS)]
    in_maps = []
    for cid in range(n_cores):
        r, c = divmod(cid, D_GROUPS)
        in_maps.append({
            "xtp": xts[r],
            "w1p": w1s[c],
            "w2p": w2s[c],
            "b1s": b1s[c],
            "b2s": b2s[c],
        })
    return in_maps


def unshard_output(results, b_dim, dout):
    bs = b_dim // B_GROUPS
    ds = dout // D_GROUPS
    out = np.empty((b_dim, dout), dtype=np.float32)
    for cid in range(N_CORES):
        r, c = divmod(cid, D_GROUPS)
        out[r * bs:(r + 1) * bs, c * ds:(c + 1) * ds] = results[cid]["out"]
    return out


_PROGRAM_CACHE = {}


def _get_program(din, b, dout_s, k, with_b1, with_b2):
    key = (din, b, dout_s, k, with_b1, with_b2, CH)
    if key not in _PROGRAM_CACHE:
        _PROGRAM_CACHE[key] = build_program(
            din, b, dout_s, k, with_b1, with_b2, ch=CH)
    return _PROGRAM_CACHE[key]


class ParallelRunner:
    """Dispatch the per-core NEFF to each NeuronCore via its own jit so the
    8 executions overlap. (run_bass_kernel_spmd's shard_map path serializes
    the per-device executes through the axon proxy — measured 8x slower
    wall-clock for identical device work.)"""

    def __init__(self, nc, n_cores=N_CORES):
        import jax
        from concourse import bass2jax
        bass2jax.install_neuronx_cc_hook()
        self.jax = jax
        self.n_cores = n_cores
        part = nc.partition_id_tensor.name if nc.partition_id_tensor else None

        in_names, out_names, out_avals, zero_outs = [], [], [], []
        for alloc in nc.m.functions[0].allocations:
            if not isinstance(alloc, mybir.MemoryLocationSet):
                continue
            name = alloc.memorylocations[0].name
            if alloc.kind == "ExternalInput":
                if name != part:
                    in_names.append(name)
            elif alloc.kind == "ExternalOutput":
                out_names.append(name)
                shape = tuple(alloc.tensor_shape)
                dtype = mybir.dt.np(alloc.dtype)
                out_avals.append(jax.core.ShapedArray(shape, dtype))
                zero_outs.append(np.zeros(shape, dtype))
        self.in_names, self.out_names = in_names, out_names
        all_names = in_names + out_names + ([part] if part else [])

        def _body(*args):
            operands = list(args)
            if part is not None:
                operands.append(bass2jax.partition_id_tensor())
            return tuple(bass2jax._bass_exec_p.bind(
                *operands,
                out_avals=tuple(out_avals),
                in_names=tuple(all_names),
                out_names=tuple(out_names),
                lowering_input_output_aliases=(),
                sim_require_finite=True,
                sim_require_nnan=True,
                nc=nc,
            ))

        self.devices = jax.devices()[:n_cores]
        self.fns = [jax.jit(_body, device=d, keep_unused=True)
                    for d in self.devices]
        self.zero_dev = [
            [jax.device_put(z, d) for z in zero_outs] for d in self.devices]

    def __call__(self, in_maps):
        outs = []
        for c in range(self.n_cores):
            args = [self.jax.device_put(np.asarray(in_maps[c][n]),
                                        self.devices[c])
                    for n in self.in_names]
            outs.append(self.fns[c](*args, *self.zero_dev[c]))
        self.jax.block_until_ready(outs)
        return [{n: np.asarray(outs[c][i])
                 for i, n in enumerate(self.out_names)}
                for c in range(self.n_cores)]


_RUNNER_CACHE = {}


def _run(nc, in_maps):
    key = id(nc)
    try:
        if key not in _RUNNER_CACHE:
            _RUNNER_CACHE[key] = ParallelRunner(nc)
        return _RUNNER_CACHE[key](in_maps)
    except Exception:
        res = run_bass_kernel_spmd(nc, in_maps,
                                   core_ids=list(range(N_CORES)))
        return res.results


def kernel(x, w1, b1, w2, b2):
    x = np.asarray(x, dtype=np.float32)
    w1 = np.asarray(w1, dtype=np.float32)
    b1 = np.asarray(b1, dtype=np.float32)
    w2 = np.asarray(w2, dtype=np.float32)
    b2 = np.asarray(b2, dtype=np.float32)

    b_dim, din = x.shape
    _, dout, k = w1.shape
    bs = b_dim // B_GROUPS
    ds = dout // D_GROUPS

    nc = _get_program(din, bs, ds, k,
                      bool(np.any(b1)), bool(np.any(b2)))
    in_maps = shard_inputs(x, w1, b1, w2, b2)
    results = _run(nc, in_maps)
    return np.ascontiguousarray(unshard_output(results, b_dim, dout))


# revision 16
# speedup vs baseline: 1.0899x; 1.0054x over previous
"""TRN2 Bass kernel for nn_Mix2Layer (dense MLP mixture).

Reference computation (all fp32):
    g   = relu(einsum('bi,iok->bok', x, w1) + b1)        # [B, DOUT, K]
    out = einsum('bi,iok,bok->bo', x, w2, g) + b2        # [B, DOUT]

Strategy: 2x4 grid over the 8 NeuronCores — batch B split in 2 groups of
1024 rows, DOUT split in 4 shards of 512 (the bok intermediate never
leaves its core). On each core both einsums are plain matmuls of the
core's x rows [1024, DIN] against the shard's weights flattened to
[DIN, DS*K], run on the PE array in float32r — the PE fast path for
4-byte floats (1 cycle/row when the moving dim is >=256, i.e.
bf16-speed). float32r keeps 11 explicit mantissa bits (measured on
hardware: round-to-nearest-even at 11 bits on both operands reproduces
the PE result to 1e-7), giving ~2e-4 relative error overall.

All operands are pre-rounded to the fp32r grid and PACKED on the HOST
into per-tile contiguous blocks (xT: one 1 MB block per b-tile, w1/w2:
one 4 MB block per ok-chunk), so every DMA reads 8-32 KB contiguous per
partition at HBM line rate. All input DMAs go through the sync-engine
HWDGE ring in consumption order — a second concurrent DMA stream
(scalar ring) measurably starves (~70 MB/s) against the main stream and
stalled the PE for tens of us waiting on xT tiles.

Inner loop (ch=512 columns of the flattened DS*K=8192, h1 staged through
SBUF to halve the matmul instruction count — N=512 moving dim amortizes
the ~11 ns per-matmul issue overhead measured at N=256):
  for ok_chunk (16 x 512 cols):
    phase A: for b_tile (8): psum_h1 = sum_i xT_i.T @ w1_chunk_i
             h1s[b_tile] = relu(psum_h1)          (ScalarE -> SBUF)
    phase B: for b_tile (8): psum_h2 = sum_i xT_i.T @ w2_chunk_i
             p = h1s[b_tile] * psum_h2            (VectorE, one fused op)
             acc[b_tile][:, chunk] = reduce_k(p)  (VectorE, 3D-AP reduce)
  DMA acc tiles -> out rows
"""
import numpy as np

import concourse.bass as bass
import concourse.tile as tile
import concourse.mybir as mybir
from concourse import bacc
from concourse.bass_interp import get_hw_module
from concourse.bass_utils import run_bass_kernel_spmd

P = 128
f32 = mybir.dt.float32
f32r = mybir.dt.float32r

N_CORES = 8
B_GROUPS = 2   # batch split across cores
D_GROUPS = 4   # dout split across cores
F32R_MANT_BITS = 11


def round_f32r(a):
    """Round fp32 array to the fp32r grid (11 explicit mantissa bits, RNE)."""
    a = np.ascontiguousarray(a, dtype=np.float32)
    bits = a.view(np.uint32)
    shift = np.uint32(23 - F32R_MANT_BITS)
    lsb = np.uint32(1) << shift
    half = lsb >> np.uint32(1)
    rounded = (bits + half + ((bits >> shift) & np.uint32(1)) - np.uint32(1)) \
        & ~np.uint32(lsb - np.uint32(1))
    return rounded.view(np.float32)


def build_program(din, b, dout_s, k, with_b1, with_b2, ch=512,
                  num_devices=N_CORES):
    """Build + schedule + compile the per-core Bass program.

    din: contraction dim; b: per-core batch rows; dout_s: per-core dout
    shard; k: mixture. ch: ok-chunk width (matmul free dim).
    """
    okw = dout_s * k
    assert din % P == 0 and b % P == 0 and okw % ch == 0
    assert ch % k == 0
    it_n = din // P
    nbt = b // P
    nch = okw // ch
    o_ch = ch // k

    nc = bacc.Bacc("TRN2", target_bir_lowering=False, debug=False,
                   enable_asserts=True, num_devices=num_devices)
    # Host-packed layouts: one contiguous [P, cols] block per tile.
    xt_d = nc.dram_tensor("xtp", [nbt, P, it_n * P], f32r,
                          kind="ExternalInput").ap()
    w1_d = nc.dram_tensor("w1p", [nch, P, it_n * ch], f32r,
                          kind="ExternalInput").ap()
    w2_d = nc.dram_tensor("w2p", [nch, P, it_n * ch], f32r,
                          kind="ExternalInput").ap()
    b1_d = nc.dram_tensor("b1s", [okw], f32r, kind="ExternalInput").ap()
    b2_d = nc.dram_tensor("b2s", [dout_s], f32, kind="ExternalInput").ap()
    out_d = nc.dram_tensor("out", [b, dout_s], f32, kind="ExternalOutput").ap()

    from contextlib import ExitStack
    with tile.TileContext(nc) as tc, ExitStack() as ctx:
        # (A PE warm-up block — scratch matmuls during the DMA preamble to
        # pre-flip the HAM clock gate — was tried and reverted: N=64 fp32
        # scratch matmuls don't register as sustained PE activity in the
        # HAM window, the gate still flipped ~13 us into the real stream,
        # and the net effect was +3 us. The cold-ramp cost it targeted is
        # only ~3.6 us.)
        xt_pool = ctx.enter_context(tc.tile_pool(name="xt", bufs=nbt))
        # Single shared-tag ring: at most one weight DMA in flight at a
        # time behind the current pair — two concurrent DMA write streams
        # into SBUF degrade the PE issue rate (120 -> 144 ns measured).
        w_pool = ctx.enter_context(tc.tile_pool(name="w", bufs=3))
        h1_pool = ctx.enter_context(tc.tile_pool(name="h1", bufs=nbt))
        acc_pool = ctx.enter_context(tc.tile_pool(name="acc", bufs=nbt))
        ep_pool = ctx.enter_context(tc.tile_pool(name="ep", bufs=3))
        const_pool = ctx.enter_context(tc.tile_pool(name="const", bufs=1))
        ps1_pool = ctx.enter_context(
            tc.tile_pool(name="ps1", bufs=2, space="PSUM"))
        ps2_pool = ctx.enter_context(
            tc.tile_pool(name="ps2", bufs=2, space="PSUM"))

        if with_b1:
            ones_t = const_pool.tile([1, P], f32r, tag="ones")
            nc.any.memset(ones_t[:], 1.0)
        if with_b2:
            b2bc = const_pool.tile([P, dout_s], f32, tag="b2bc")
            nc.gpsimd.dma_start(b2bc[:],
                                b2_d[None, :].broadcast_to([P, dout_s]))

        # Issue order on the single sync ring follows first-use order:
        # xt0, w1c0, xt1..7, w2c0, then the steady chunk stream. Delivery
        # (~2.8 us per xt tile at line rate) stays ahead of phase-A
        # consumption (~3.6 us per b-tile group).
        xts = [xt_pool.tile([P, it_n * P], f32r, tag="xtb", name=f"xt_{i}")
               for i in range(nbt)]
        nc.sync.dma_start(xts[0][:], xt_d[0])

        w1_t0 = w_pool.tile([P, it_n * ch], f32r, tag="w")
        nc.sync.dma_start(w1_t0[:], w1_d[0])
        for bt in range(1, nbt):
            nc.sync.dma_start(xts[bt][:], xt_d[bt])
        w2_t0 = w_pool.tile([P, it_n * ch], f32r, tag="w")
        nc.sync.dma_start(w2_t0[:], w2_d[0])

        h1s = [h1_pool.tile([P, ch], f32, tag="h1s", name=f"h1_{i}")
               for i in range(nbt)]
        accs = [acc_pool.tile([P, dout_s], f32, tag="acc",
                              name=f"acc_{i}")
                for i in range(nbt)]

        for c in range(nch):
            if c == 0:
                w1_t, w2_t = w1_t0, w2_t0
            else:
                w1_t = w_pool.tile([P, it_n * ch], f32r, tag="w")
                nc.sync.dma_start(w1_t[:], w1_d[c])
                w2_t = w_pool.tile([P, it_n * ch], f32r, tag="w")
                nc.sync.dma_start(w2_t[:], w2_d[c])
            if with_b1:
                b1r = ep_pool.tile([1, ch], f32r, tag="b1r")
                nc.sync.dma_start(
                    b1r[:], b1_d[None, c * ch:(c + 1) * ch])

            # phase A: h1 = relu(x @ w1chunk) for all b-tiles
            for bt in range(nbt):
                ph1 = ps1_pool.tile([P, ch], f32, tag="ph1")
                for it in range(it_n):
                    nc.tensor.matmul(
                        ph1[:],
                        lhsT=xts[bt][:, it * P:(it + 1) * P],
                        rhs=w1_t[:, it * ch:(it + 1) * ch],
                        start=(it == 0),
                        stop=(it == it_n - 1 and not with_b1),
                    )
                if with_b1:
                    nc.tensor.matmul(ph1[:], lhsT=ones_t[:], rhs=b1r[:],
                                     start=False, stop=True)
                nc.scalar.activation(
                    h1s[bt][:], ph1[:], mybir.ActivationFunctionType.Relu)

            # phase B: h2 = x @ w2chunk; acc[:, chunk] = reduce_k(h1 * h2)
            for bt in range(nbt):
                ph2 = ps2_pool.tile([P, ch], f32, tag="ph2")
                for it in range(it_n):
                    nc.tensor.matmul(
                        ph2[:],
                        lhsT=xts[bt][:, it * P:(it + 1) * P],
                        rhs=w2_t[:, it * ch:(it + 1) * ch],
                        start=(it == 0),
                        stop=(it == it_n - 1),
                    )
                p_t = ep_pool.tile([P, ch], f32, tag="pt")
                nc.vector.scalar_tensor_tensor(
                    out=p_t[:], in0=ph2[:], scalar=0.0, in1=h1s[bt][:],
                    op0=mybir.AluOpType.add, op1=mybir.AluOpType.mult)
                nc.vector.tensor_reduce(
                    out=accs[bt][:, c * o_ch:(c + 1) * o_ch],
                    in_=p_t[:].rearrange("p (o k) -> p o k", k=k),
                    axis=mybir.AxisListType.X,
                    op=mybir.AluOpType.add)

        for bt in range(nbt):
            if with_b2:
                nc.vector.tensor_add(accs[bt][:], accs[bt][:], b2bc[:])
            nc.scalar.dma_start(
                out_d[bt * P:(bt + 1) * P, :],
                accs[bt][:])

    nc.compile()
    nc.m = get_hw_module(nc.m)
    return nc


def _pack_xt(x_rows, it_n):
    """[b, din] fp32r rows -> [nbt, P, it_n*P]: per-b-tile contiguous,
    partition-major, so each tile is one line-rate DMA."""
    b, din = x_rows.shape
    nbt = b // P
    a = x_rows.reshape(nbt, P, it_n, P)          # [bt, bl, it, pi]
    a = a.transpose(0, 3, 2, 1)                  # [bt, pi, it, bl]
    return np.ascontiguousarray(a.reshape(nbt, P, it_n * P))


def _pack_w(w_flat, it_n, ch):
    """[din, okw] fp32r -> [nch, P, it_n*ch]: per-chunk contiguous."""
    din, okw = w_flat.shape
    nch = okw // ch
    a = w_flat.reshape(it_n, P, nch, ch)         # [it, p, c, j]
    a = a.transpose(2, 1, 0, 3)                  # [c, p, it, j]
    return np.ascontiguousarray(a.reshape(nch, P, it_n * ch))


CH = 512


def shard_inputs(x, w1, b1, w2, b2, n_cores=N_CORES):
    b_dim, din = x.shape
    _, dout, k = w1.shape
    bs = b_dim // B_GROUPS
    ds = dout // D_GROUPS
    it_n = din // P
    xr = round_f32r(np.asarray(x, np.float32))
    xts = [_pack_xt(xr[r * bs:(r + 1) * bs], it_n) for r in range(B_GROUPS)]
    w1s = [_pack_w(round_f32r(w1[:, c * ds:(c + 1) * ds, :])
                   .reshape(din, ds * k), it_n, CH)
           for c in range(D_GROUPS)]
    w2s = [_pack_w(round_f32r(w2[:, c * ds:(c + 1) * ds, :])
                   .reshape(din, ds * k), it_n, CH)
           for c in range(D_GROUPS)]
    b1s = [round_f32r(b1[c * ds:(c + 1) * ds, :]).reshape(ds * k)
           for c in range(D_GROUPS)]
    b2s = [np.ascontiguousarray(b2[c * ds:(c + 1) * ds], dtype=np.float32)
           for c in range(D_GROUPS)]
    in_maps = []
    for cid in range(n_cores):
        r, c = divmod(cid, D_GROUPS)
        in_maps.append({
            "xtp": xts[r],
            "w1p": w1s[c],
            "w2p": w2s[c],
            "b1s": b1s[c],
            "b2s": b2s[c],
        })
    return in_maps


def unshard_output(results, b_dim, dout):
    bs = b_dim // B_GROUPS
    ds = dout // D_GROUPS
    out = np.empty((b_dim, dout), dtype=np.float32)
    for cid in range(N_CORES):
        r, c = divmod(cid, D_GROUPS)
        out[r * bs:(r + 1) * bs, c * ds:(c + 1) * ds] = results[cid]["out"]
    return out


_PROGRAM_CACHE = {}


def _get_program(din, b, dout_s, k, with_b1, with_b2):
    key = (din, b, dout_s, k, with_b1, with_b2, CH)
    if key not in _PROGRAM_CACHE:
        _PROGRAM_CACHE[key] = build_program(
            din, b, dout_s, k, with_b1, with_b2, ch=CH)
    return _PROGRAM_CACHE[key]


class ParallelRunner:
    """Dispatch the per-core NEFF to each NeuronCore via its own jit so the
    8 executions overlap. (run_bass_kernel_spmd's shard_map path serializes
    the per-device executes through the axon proxy — measured 8x slower
    wall-clock for identical device work.)"""

    def __init__(self, nc, n_cores=N_CORES):
        import jax
        from concourse import bass2jax
        bass2jax.install_neuronx_cc_hook()
        self.jax = jax
        self.n_cores = n_cores
        part = nc.partition_id_tensor.name if nc.partition_id_tensor else None

        in_names, out_names, out_avals, zero_outs = [], [], [], []
        for alloc in nc.m.functions[0].allocations:
            if not isinstance(alloc, mybir.MemoryLocationSet):
                continue
            name = alloc.memorylocations[0].name
            if alloc.kind == "ExternalInput":
                if name != part:
                    in_names.append(name)
            elif alloc.kind == "ExternalOutput":
                out_names.append(name)
                shape = tuple(alloc.tensor_shape)
                dtype = mybir.dt.np(alloc.dtype)
                out_avals.append(jax.core.ShapedArray(shape, dtype))
                zero_outs.append(np.zeros(shape, dtype))
        self.in_names, self.out_names = in_names, out_names
        all_names = in_names + out_names + ([part] if part else [])

        def _body(*args):
            operands = list(args)
            if part is not None:
                operands.append(bass2jax.partition_id_tensor())
            return tuple(bass2jax._bass_exec_p.bind(
                *operands,
                out_avals=tuple(out_avals),
                in_names=tuple(all_names),
                out_names=tuple(out_names),
                lowering_input_output_aliases=(),
                sim_require_finite=True,
                sim_require_nnan=True,
                nc=nc,
            ))

        self.devices = jax.devices()[:n_cores]
        self.fns = [jax.jit(_body, device=d, keep_unused=True)
                    for d in self.devices]
        self.zero_dev = [
            [jax.device_put(z, d) for z in zero_outs] for d in self.devices]

    def __call__(self, in_maps):
        outs = []
        for c in range(self.n_cores):
            args = [self.jax.device_put(np.asarray(in_maps[c][n]),
                                        self.devices[c])
                    for n in self.in_names]
            outs.append(self.fns[c](*args, *self.zero_dev[c]))
        self.jax.block_until_ready(outs)
        return [{n: np.asarray(outs[c][i])
                 for i, n in enumerate(self.out_names)}
                for c in range(self.n_cores)]


_RUNNER_CACHE = {}


def _run(nc, in_maps):
    key = id(nc)
    try:
        if key not in _RUNNER_CACHE:
            _RUNNER_CACHE[key] = ParallelRunner(nc)
        return _RUNNER_CACHE[key](in_maps)
    except Exception:
        res = run_bass_kernel_spmd(nc, in_maps,
                                   core_ids=list(range(N_CORES)))
        return res.results


def kernel(x, w1, b1, w2, b2):
    x = np.asarray(x, dtype=np.float32)
    w1 = np.asarray(w1, dtype=np.float32)
    b1 = np.asarray(b1, dtype=np.float32)
    w2 = np.asarray(w2, dtype=np.float32)
    b2 = np.asarray(b2, dtype=np.float32)

    b_dim, din = x.shape
    _, dout, k = w1.shape
    bs = b_dim // B_GROUPS
    ds = dout // D_GROUPS

    nc = _get_program(din, bs, ds, k,
                      bool(np.any(b1)), bool(np.any(b2)))
    in_maps = shard_inputs(x, w1, b1, w2, b2)
    results = _run(nc, in_maps)
    return np.ascontiguousarray(unshard_output(results, b_dim, dout))
